# revision 6
# baseline (speedup 1.0000x reference)
"""Trainium2 Bass kernel for batched dot-product attention scores + softmax.

hidden: [1, 32, 1024] f32, encoder_outputs: [4096, 32, 1024] f32
out[b, 0, l] = softmax_l( sum_h hidden[0,b,h] * encoder_outputs[l,b,h] )

Sharding: batch dim (32) split 4-per-core across 8 NeuronCores (pure data
parallel). Each core streams its 64 MiB encoder_outputs shard once.

Per-core plan (B=4 local batches, L=4096, H=1024, P=128 partitions):
  - hidden shard broadcast to all 128 partitions once: hb [128, 4, 1024].
  - 32 l-tiles: DMA et [128, 4, 1024] (2 MiB contiguous), then per batch a
    fused DVE tensor_tensor_reduce: prod = et*hb, scores[:,b,i] = sum_h.
    One pass over the data on DVE (fp32 TT is 1x mode) stays under the
    ~187us/core HBM roofline.
  - Softmax with partition-local stats + gpsimd partition_all_reduce for
    the cross-partition max/sum (flash-style rescale, exact in exact
    arithmetic).
  - DVE 32x32 stream-transposes so the store to HBM has contiguous runs.
"""

import numpy as np

L = 4096
B_TOTAL = 32
H = 1024
N_CORES = 8
B = B_TOTAL // N_CORES  # 4 local batches per core
P = 128
NT = L // P  # 32 l-tiles

_CACHE = {}


def _body(tc, e_ap, h_ap, o_ap):
    import concourse.bass as bass
    from concourse import mybir, bass_isa

    nc = tc.nc
    f32 = mybir.dt.float32
    Alu = mybir.AluOpType
    Act = mybir.ActivationFunctionType

    # [4096, 4, 1024] -> [32, 128, 4, 1024]
    e_r = e_ap.rearrange("(n p) b h -> n p b h", p=P)

    with (
        tc.tile_pool(name="consts", bufs=1) as consts,
        tc.tile_pool(name="epool", bufs=4) as epool,
        tc.tile_pool(name="scratch", bufs=1) as scratch,
        tc.tile_pool(name="small", bufs=1) as small,
    ):
        # hidden shard replicated across all 128 partitions.
        hb = consts.tile([P, B, H], f32)
        h_bcast = bass.AP(
            tensor=h_ap.tensor,
            offset=h_ap.offset,
            ap=[[0, P]] + list(h_ap.ap),
        )
        nc.gpsimd.dma_start(out=hb[:], in_=h_bcast)

        scores = small.tile([P, B, NT], f32)
        prod = scratch.tile([P, B, H], f32)  # TT-reduce main output (discarded)

        for i in range(NT):
            et = epool.tile([P, B, H], f32)
            nc.sync.dma_start(out=et[:], in_=e_r[i])
            for b in range(B):
                # out = (et * 1.0) * hb, accum_out = sum(out) — one fused
                # DVE pass (tensor_tensor_reduce opcode is rejected by this
                # runtime; InstTensorScalarPtr works).
                nc.vector.scalar_tensor_tensor(
                    out=prod[:, b, :],
                    in0=et[:, b, :],
                    scalar=1.0,
                    in1=hb[:, b, :],
                    op0=Alu.mult,
                    op1=Alu.mult,
                    accum_out=scores[:, b, i : i + 1],
                )

        # ---- softmax over all 4096 scores per batch ----
        # scores[p, b, i] holds score at l = 128*i + p.
        mst = small.tile([P, B], f32)      # per-partition max
        negm = small.tile([P, B], f32)
        eexp = small.tile([P, B, NT], f32)  # exp(s - m_p)
        ssum = small.tile([P, B], f32)     # per-partition sum of eexp
        mall = small.tile([P, B], f32)     # global max (replicated)
        negmall = small.tile([P, B], f32)
        wt = small.tile([P, B], f32)       # exp(m_p - M)
        swt = small.tile([P, B], f32)      # ssum * wt
        zt = small.tile([P, B], f32)       # global sum (replicated)
        rzt = small.tile([P, B], f32)      # 1/Z
        alphat = small.tile([P, B], f32)   # wt / Z
        attn = small.tile([P, B, NT], f32)
        outt = small.tile([P, P], f32)     # transposed output staging

        for b in range(B):
            nc.vector.reduce_max(
                out=mst[:, b : b + 1], in_=scores[:, b, :], axis=mybir.AxisListType.X
            )
        nc.vector.tensor_scalar_mul(negm[:], mst[:], -1.0)
        for b in range(B):
            nc.scalar.activation(
                out=eexp[:, b, :],
                in_=scores[:, b, :],
                func=Act.Exp,
                bias=negm[:, b : b + 1],
                scale=1.0,
                accum_out=ssum[:, b : b + 1],
            )
        nc.gpsimd.partition_all_reduce(
            mall[:], mst[:], channels=P, reduce_op=bass_isa.ReduceOp.max
        )
        nc.vector.tensor_scalar_mul(negmall[:], mall[:], -1.0)
        for b in range(B):
            nc.scalar.activation(
                out=wt[:, b : b + 1],
                in_=mst[:, b : b + 1],
                func=Act.Exp,
                bias=negmall[:, b : b + 1],
                scale=1.0,
            )
        nc.vector.tensor_mul(swt[:], wt[:], ssum[:])
        nc.gpsimd.partition_all_reduce(
            zt[:], swt[:], channels=P, reduce_op=bass_isa.ReduceOp.add
        )
        nc.vector.reciprocal(rzt[:], zt[:])
        nc.vector.tensor_mul(alphat[:], wt[:], rzt[:])
        for b in range(B):
            nc.vector.tensor_scalar_mul(
                attn[:, b, :], eexp[:, b, :], alphat[:, b : b + 1]
            )

        # Transpose 32x32 blocks so HBM store has 128B contiguous runs.
        # outt[32j + c, 32b + p'] = attn[32j + p', b, c] = value at
        # l = 128*c + 32*j + p' for batch b.
        for b in range(B):
            for j in range(P // 32):
                nc.vector.transpose(
                    out=outt[32 * j : 32 * j + 32, 32 * b : 32 * b + 32],
                    in_=attn[32 * j : 32 * j + 32, b, :],
                )
        # DRAM view enumerating (j, c, b, p) to match outt's layout.
        o_r = o_ap.rearrange("b (c j p) -> j c b p", c=32, j=P // 32, p=32)
        for j in range(P // 32):
            nc.sync.dma_start(out=o_r[j], in_=outt[32 * j : 32 * j + 32, :])


def _build():
    import concourse.bacc as bacc
    import concourse.tile as tile
    from concourse import mybir

    nc = bacc.Bacc("TRN2", target_bir_lowering=False, debug=False, num_devices=N_CORES)
    e = nc.dram_tensor("e", [L, B, H], mybir.dt.float32, kind="ExternalInput")
    h = nc.dram_tensor("h", [B, H], mybir.dt.float32, kind="ExternalInput")
    o = nc.dram_tensor("o", [B, L], mybir.dt.float32, kind="ExternalOutput")
    with tile.TileContext(nc) as tc:
        _body(tc, e.ap(), h.ap(), o.ap())
    nc.compile()
    return nc


def _get_nc():
    if "nc" not in _CACHE:
        _CACHE["nc"] = _build()
    return _CACHE["nc"]


def make_in_maps(hidden, encoder_outputs):
    hidden = np.asarray(hidden, dtype=np.float32)
    encoder_outputs = np.asarray(encoder_outputs, dtype=np.float32)
    in_maps = []
    for c in range(N_CORES):
        b0 = c * B
        in_maps.append(
            {
                "e": np.ascontiguousarray(encoder_outputs[:, b0 : b0 + B, :]),
                "h": np.ascontiguousarray(hidden[0, b0 : b0 + B, :]),
            }
        )
    return in_maps


def kernel(hidden, encoder_outputs, **run_kwargs):
    from concourse import bass_utils

    nc = _get_nc()
    in_maps = make_in_maps(hidden, encoder_outputs)
    res = bass_utils.run_bass_kernel_spmd(
        nc, in_maps, core_ids=list(range(N_CORES)), **run_kwargs
    )
    out = np.concatenate([res.results[c]["o"] for c in range(N_CORES)], axis=0)
    _CACHE["last_results"] = res
    return out[:, None, :].astype(np.float32)


# revision 12
# speedup vs baseline: 23.9983x; 23.9983x over previous
"""Trainium2 Bass kernel for batched dot-product attention scores + softmax.

hidden: [1, 32, 1024] f32, encoder_outputs: [4096, 32, 1024] f32
out[b, 0, l] = softmax_l( sum_h hidden[0,b,h] * encoder_outputs[l,b,h] )

Sharding: batch dim (32) split 4-per-core across 8 NeuronCores (pure data
parallel). Each core streams its 64 MiB encoder_outputs shard once.

Per-core plan (B=4 local batches, L=4096, H=1024, P=128 partitions):
  - hidden shard broadcast to all 128 partitions once: hb [128, 4, 1024].
  - 32 l-tiles: DMA et [128, 4, 1024] (2 MiB contiguous), then per batch a
    fused DVE tensor_tensor_reduce: prod = et*hb, scores[:,b,i] = sum_h.
    One pass over the data on DVE (fp32 TT is 1x mode) stays under the
    ~187us/core HBM roofline.
  - Softmax with partition-local stats + gpsimd partition_all_reduce for
    the cross-partition max/sum (flash-style rescale, exact in exact
    arithmetic).
  - DVE 32x32 stream-transposes so the store to HBM has contiguous runs.
"""

import numpy as np

L = 4096
B_TOTAL = 32
H = 1024
N_CORES = 8
B = B_TOTAL // N_CORES  # 4 local batches per core
P = 128
NT = L // P  # 32 l-tiles

_CACHE = {}


def _body(tc, e_ap, h_ap, o_ap, reps=1):
    import concourse.bass as bass
    from concourse import mybir, bass_isa

    nc = tc.nc
    f32 = mybir.dt.float32
    Alu = mybir.AluOpType
    Act = mybir.ActivationFunctionType

    # [4096, 4, 1024] -> [32, 128, 4, 1024]
    e_r = e_ap.rearrange("(n p) b h -> n p b h", p=P)

    with (
        tc.tile_pool(name="consts", bufs=1) as consts,
        tc.tile_pool(name="epool", bufs=6) as epool,
        tc.tile_pool(name="scratch", bufs=1) as scratch,
        tc.tile_pool(name="small", bufs=1) as small,
    ):
        # hidden shard replicated across all 128 partitions; one DMA per
        # batch so the first STT only waits for its own batch's row.
        hb = consts.tile([P, B, H], f32)
        for b in range(B):
            h_row = h_ap[b : b + 1, :]
            h_bcast = bass.AP(
                tensor=h_row.tensor,
                offset=h_row.offset,
                ap=[[0, P]] + [list(h_row.ap[-1])],
            )
            nc.gpsimd.dma_start(out=hb[:, b, :], in_=h_bcast)

        # Warm the ACT Exp spline table while the kernel is DMA-bound so the
        # softmax tail doesn't pay the ~2.7us table load.
        warm = consts.tile([P, 1], f32)
        nc.vector.memset(warm[:], 0.0)
        nc.scalar.activation(out=warm[:], in_=warm[:], func=Act.Exp)

        for _rep in range(reps):
            _rep_body(tc, e_r, o_ap, hb, epool, scratch, small)


def _rep_body(tc, e_r, o_ap, hb, epool, scratch, small):
    import concourse.bass as bass
    from concourse import mybir, bass_isa

    nc = tc.nc
    f32 = mybir.dt.float32
    Alu = mybir.AluOpType
    Act = mybir.ActivationFunctionType

    if True:  # keep indentation of original body
        scores = small.tile([P, B, NT], f32)
        prod = scratch.tile([P, B, H], f32)  # TT-reduce main output (discarded)

        for i in range(NT):
            et = epool.tile([P, B, H], f32)
            nc.sync.dma_start(out=et[:], in_=e_r[i])
            for b in range(B):
                # out = (et * 1.0) * hb, accum_out = sum(out) — one fused
                # DVE pass (tensor_tensor_reduce opcode is rejected by this
                # runtime; InstTensorScalarPtr works).
                nc.vector.scalar_tensor_tensor(
                    out=prod[:, b, :],
                    in0=et[:, b, :],
                    scalar=1.0,
                    in1=hb[:, b, :],
                    op0=Alu.mult,
                    op1=Alu.mult,
                    accum_out=scores[:, b, i : i + 1],
                )

        # ---- softmax over all 4096 scores per batch ----
        # scores[p, b, i] holds score at l = 128*i + p.
        mst = small.tile([P, B], f32)      # per-partition max
        negm = small.tile([P, B], f32)
        eexp = small.tile([P, B, NT], f32)  # exp(s - m_p)
        ssum = small.tile([P, B], f32)     # per-partition sum of eexp
        mall = small.tile([P, B], f32)     # global max (replicated)
        negmall = small.tile([P, B], f32)
        wt = small.tile([P, B], f32)       # exp(m_p - M)
        swt = small.tile([P, B], f32)      # ssum * wt
        zt = small.tile([P, B], f32)       # global sum (replicated)
        rzt = small.tile([P, B], f32)      # 1/Z
        alphat = small.tile([P, B], f32)   # wt / Z
        attn = small.tile([P, B, NT], f32)
        outt = small.tile([P, P], f32)     # transposed output staging

        for b in range(B):
            nc.vector.reduce_max(
                out=mst[:, b : b + 1], in_=scores[:, b, :], axis=mybir.AxisListType.X
            )
        nc.vector.tensor_scalar_mul(negm[:], mst[:], -1.0)
        for b in range(B):
            nc.scalar.activation(
                out=eexp[:, b, :],
                in_=scores[:, b, :],
                func=Act.Exp,
                bias=negm[:, b : b + 1],
                scale=1.0,
                accum_out=ssum[:, b : b + 1],
            )
        nc.gpsimd.partition_all_reduce(
            mall[:], mst[:], channels=P, reduce_op=bass_isa.ReduceOp.max
        )
        nc.vector.tensor_scalar_mul(negmall[:], mall[:], -1.0)
        for b in range(B):
            nc.scalar.activation(
                out=wt[:, b : b + 1],
                in_=mst[:, b : b + 1],
                func=Act.Exp,
                bias=negmall[:, b : b + 1],
                scale=1.0,
            )
        nc.vector.tensor_mul(swt[:], wt[:], ssum[:])
        nc.gpsimd.partition_all_reduce(
            zt[:], swt[:], channels=P, reduce_op=bass_isa.ReduceOp.add
        )
        nc.vector.reciprocal(rzt[:], zt[:])
        nc.vector.tensor_mul(alphat[:], wt[:], rzt[:])
        for b in range(B):
            nc.vector.tensor_scalar_mul(
                attn[:, b, :], eexp[:, b, :], alphat[:, b : b + 1]
            )

        # Transpose 32x32 blocks so HBM store has 128B contiguous runs.
        # outt[32j + c, 32b + p'] = attn[32j + p', b, c] = value at
        # l = 128*c + 32*j + p' for batch b. j-major so each partition
        # block's store can launch as soon as its 4 transposes finish.
        # DRAM view enumerates (j, c, b, p) to match outt's layout.
        o_r = o_ap.rearrange("b (c j p) -> j c b p", c=32, j=P // 32, p=32)
        for j in range(P // 32):
            for b in range(B):
                nc.vector.transpose(
                    out=outt[32 * j : 32 * j + 32, 32 * b : 32 * b + 32],
                    in_=attn[32 * j : 32 * j + 32, b, :],
                )
            nc.sync.dma_start(out=o_r[j], in_=outt[32 * j : 32 * j + 32, :])


def _build(reps=1):
    import concourse.bacc as bacc
    import concourse.tile as tile
    from concourse import mybir

    nc = bacc.Bacc("TRN2", target_bir_lowering=False, debug=False, num_devices=N_CORES)
    e = nc.dram_tensor("e", [L, B, H], mybir.dt.float32, kind="ExternalInput")
    h = nc.dram_tensor("h", [B, H], mybir.dt.float32, kind="ExternalInput")
    o = nc.dram_tensor("o", [B, L], mybir.dt.float32, kind="ExternalOutput")
    with tile.TileContext(nc) as tc:
        _body(tc, e.ap(), h.ap(), o.ap(), reps=reps)
    nc.compile()
    return nc


def _get_nc(reps=1):
    key = f"nc{reps}"
    if key not in _CACHE:
        _CACHE[key] = _build(reps=reps)
    return _CACHE[key]


def make_in_maps(hidden, encoder_outputs):
    hidden = np.asarray(hidden, dtype=np.float32)
    encoder_outputs = np.asarray(encoder_outputs, dtype=np.float32)
    in_maps = []
    for c in range(N_CORES):
        b0 = c * B
        in_maps.append(
            {
                "e": np.ascontiguousarray(encoder_outputs[:, b0 : b0 + B, :]),
                "h": np.ascontiguousarray(hidden[0, b0 : b0 + B, :]),
            }
        )
    return in_maps


def kernel(hidden, encoder_outputs, **run_kwargs):
    from concourse import bass_utils

    nc = _get_nc()
    in_maps = make_in_maps(hidden, encoder_outputs)
    res = bass_utils.run_bass_kernel_spmd(
        nc, in_maps, core_ids=list(range(N_CORES)), **run_kwargs
    )
    out = np.concatenate([res.results[c]["o"] for c in range(N_CORES)], axis=0)
    _CACHE["last_results"] = res
    return out[:, None, :].astype(np.float32)
